# revision 17
# baseline (speedup 1.0000x reference)
"""Trainium2 Bass kernel for nn_CrossAttention_28183575396415.

The reference block-mask gives every query exactly one key (kv = q_idx // 3),
so the softmax weight is identically 1 and the q/k projections, RMSNorm and
RoPE are dead code.  The module reduces to

    out[b, t] = x_kv[b, t // 3] @ Wv.T @ Wproj.T
              = x_kv[b, t // 3] @ WfT          with WfT = Wv.T @ Wproj.T

Strategy (8 NeuronCores, SPMD):
  - Host folds the two projection matrices into WfT (computed in float64).
  - The 4*2048 = 8192 kv rows are row-sharded 8 ways (1024 rows/core).
    Each core's shard is pre-transposed on host so every device DMA is a
    natural contiguous load; the shard and the weight are concatenated into
    one [1024(k), 2048] bf16 input:
        xw[:, :1024]  = x_shard.T   (k on partitions = contraction dim)
        xw[:, 1024:]  = WfT
  - Schedule (timings from the 46.1us baseline trace):
    * ~11 warm-up matmuls on a memset scratch tile run while the input
      streams in, so the PE_HAM clock gate reaches 8/8 (2.4 GHz) before the
      first real matmul - the baseline paid ~1.7us of 1.2 GHz cold matmuls.
    * k-tile 0 is split across both HWDGE rings (x part on sync, W part on
      scalar) so the first real matmul's 384KB dependency becomes two
      concurrent 256/128KB transfers and the PE starts ~1us sooner.
    * Pass 0 (cc half 0, rows m0-m5) runs k-major in lockstep with the
      input stream; at the last k-tile each row's eviction + 3x-replicated
      output store issues immediately, so the output DMA stream starts
      right as the input stream ends (the baseline's all-rows k-major pass
      started stores 11us later, leaving a 10.8us un-overlapped tail).
    * Bridge (cc0, m6-m7) + pass 1 (cc1, all rows) run m-major, one tile
      retiring every ~1.7us, keeping the output ring saturated to the end.
  - Each z tile is written to HBM with a single DMA whose source AP repeats
    the tile 3x (stride-0 middle dim) - the t//3 replication - giving this
    core's contiguous [3072, 1024] slice of the flattened output in bf16.
  - Host unshard = concatenate the 8 slices and upcast to float32.
"""

import json
import os

import numpy as np

import concourse.bass as bass
import concourse.mybir as mybir
from bass_rust import AP
from concourse.tile import TileContext
from concourse.vector_clock import ScopedClock
from concourse.bass_utils import run_bass_kernel_spmd

P = 128          # partitions
C = 1024         # model dim
K_T = C // P     # k tiles
M_T = C // P     # row tiles per core shard
N = 512          # matmul free dim (one PSUM bank of fp32)
L = 3            # replication factor (Tq // Tkv)
ROWS_PER_CORE = 1024
N_CORES = 8
W1 = ROWS_PER_CORE + N   # x | W-cc0 columns per k-tile row block

# compute dtype: "bf16" (half the input DMA), "f32r"/"f32" for debugging
COMPUTE_DT = os.environ.get("KERNEL_COMPUTE_DT", "bf16")
# output dtype on device: "bf16" (host upcasts) or "f32"
OUT_DT = os.environ.get("KERNEL_OUT_DT", "bf16")
# PE warm-up matmuls issued before the first real matmul
N_WARM = int(os.environ.get("KERNEL_N_WARM", "30"))
# k-major lead rows in pass 0
M_LEAD = int(os.environ.get("KERNEL_M_LEAD", "6"))


class SlimTailTileContext(TileContext):
    """Tile's kernel tail is drain -> barrier -> per-semaphore clear
    instructions -> barrier.  The clears only matter if the loaded NEFF
    executes more than once; every kernel() call here builds a fresh jit
    executable (fresh NEFF load, semaphores re-initialized), so skip them
    and the second barrier.  The drain still waits for every DMA queue,
    so outputs are complete before the program ends."""

    def _drain_and_barrier(self, tick_clock, wait_clock):
        drain_inst = self.nc.sync.drain()
        wait_clock.add_sem_waits(
            drain_inst.ins, ScopedClock({None: tick_clock.global_clock})
        )
        popped = self.nc._tile_sem_poison_stack.pop()
        assert popped is self._sem_poison


def _split_multiwaits(nc: bass.Bass) -> None:
    """This container's walrus allows only ONE sync-wait on several
    instruction formats (Drain/CTRL, Matmult's LDWEIGHTS half, ...).  Tile
    can emit more.  Post-pass the serialized BIR: for any instruction with
    >1 on_wait, hoist all but the last wait onto single-wait EventSemaphore
    carriers inserted immediately before it on the same engine (waits then
    execute in queue order - semantics unchanged)."""
    raw = bass.Bass.to_json_bytes(nc)
    j = json.loads(raw)
    for f in j["functions"]:
        for bb in f["blocks"]:
            new_insts = []
            for ins in bb["instructions"]:
                si = ins.get("sync_info")
                waits = si.get("on_wait", []) if si else []
                if len(waits) > 1:
                    for i, w in enumerate(waits[:-1]):
                        carrier = {
                            "engine": ins["engine"],
                            "ins": [],
                            "outs": [],
                            "name": f"{ins['name']}_hw{i}",
                            "opcode": "EventSemaphore",
                            "sync_info": {"on_update": [], "on_wait": [w]},
                        }
                        if "debug" in ins:
                            carrier["debug"] = ins["debug"]
                        new_insts.append(carrier)
                    si["on_wait"] = waits[-1:]
                new_insts.append(ins)
            bb["instructions"] = new_insts
    patched = json.dumps(j).encode()
    nc.to_json_bytes = lambda: patched


def _rep3_src(zh_ap):
    """Source AP reading a [P, N] SBUF tile as [P, L, N] via a stride-0
    middle dim - the DMA replicates each row L times."""
    lay = zh_ap.ap
    assert len(lay) == 2, lay
    return AP(tensor=zh_ap.tensor, offset=zh_ap.offset, ap=[lay[0], [0, L], lay[1]])


def _build(compute_dt: str, out_dt: str) -> bass.Bass:
    nc = bass.Bass("TRN2")
    in_mydt = {
        "bf16": mybir.dt.bfloat16,
        "f32r": mybir.dt.float32r,
        "f32": mybir.dt.float32,
    }[compute_dt]
    out_mydt = {"bf16": mybir.dt.bfloat16, "f32": mybir.dt.float32}[out_dt]

    xw = nc.dram_tensor("xw", [C, 2 * C], in_mydt, kind="ExternalInput")
    out = nc.dram_tensor(
        "out", [L * ROWS_PER_CORE, C], out_mydt, kind="ExternalOutput"
    )
    # out row (L*g + r) <- z row g
    out_rep = out.rearrange("(g r) c -> g r c", r=L)  # [1024, L, 1024]

    with SlimTailTileContext(nc) as tc:
        with (
            tc.tile_pool(name="xw", bufs=1) as xw_pool,
            tc.tile_pool(name="psum", bufs=8, space="PSUM") as psum_pool,
            tc.tile_pool(name="zout", bufs=8) as z_pool,
        ):
            # --- scratch for PE warm-up: memset once, matmul garbage-free
            # zeros into a scratch PSUM bank.  Keeps PE_HAM's activity
            # window busy from the end of the engine preamble (~t+0.7us)
            # until the first real matmul (~t+5us) so the real matmuls run
            # at 2.4 GHz.  N=128 warm-ups are ~110ns cold, so over/under-
            # shooting the real-data arrival costs almost nothing.
            warm = xw_pool.tile([P, P], in_mydt, name="warm", tag="warm")
            nc.vector.memset(warm[:], 0.0)

            # --- input stream: each k-tile is one FULL row-block of xw
            # ([128, 2048] = x columns + both W column halves), a single
            # 512KB transfer whose HBM rows are 4KB contiguous - the
            # highest-throughput descriptor shape.  k0 is split x|W across
            # the two rings so the first matmul's dependency halves; later
            # k-tiles alternate rings.
            xwk = []
            for k in range(K_T):
                t = xw_pool.tile([P, 2 * C], in_mydt, name=f"xwk{k}", tag=f"xwk{k}")
                xwk.append(t)
            nc.sync.dma_start(xwk[0][:, :C], xw[0:P, :C])
            nc.scalar.dma_start(xwk[0][:, C:], xw[0:P, C:])
            in_eng = {1: nc.sync, 3: nc.sync, 5: nc.sync, 7: nc.sync,
                      2: nc.scalar, 4: nc.scalar, 6: nc.scalar}
            for k in range(1, K_T):
                in_eng[k].dma_start(xwk[k][:], xw[k * P : (k + 1) * P, :])

            # --- PE warm-up matmuls (independent of any DMA)
            if N_WARM:
                wps = psum_pool.tile([P, P], mybir.dt.float32, name="wps", tag="ps")
                for i in range(N_WARM):
                    nc.tensor.matmul(
                        wps[:], warm[:], warm[:], start=True, stop=True
                    )

            out_eng = [nc.sync, nc.scalar]
            n_trig = [0]

            def store_full(zh, m):
                # one DMA per row-tile: [128, 3, 1024] with 2KB-contiguous
                # HBM rows (vs 1KB for half-row stores)
                eng = out_eng[n_trig[0] % 2]
                n_trig[0] += 1
                eng.dma_start(out_rep[m * P : (m + 1) * P, :, :], _rep3_src(zh[:]))

            def evict_pair(zh, pscc0, pscc1):
                # the two PSUM banks drain on parallel engines
                nc.vector.tensor_copy(zh[:, :N], pscc0[:])
                nc.scalar.copy(zh[:, N:], pscc1[:])

            def mm(ps_cc, t, m, cc, k):
                nc.tensor.matmul(
                    ps_cc[:],
                    t[:, m * P : (m + 1) * P],
                    t[:, C + cc * N : C + (cc + 1) * N],
                    start=(k == 0),
                    stop=(k == K_T - 1),
                )

            # --- lead pass (rows m0..m{LEAD-1}, both column halves):
            # k-major in lockstep with the input stream; each row's two
            # PSUM banks accumulate across k.  At the last k-tile each
            # row's eviction + full-row store issues immediately, so the
            # output stream starts while the PE finishes the rest.
            LEAD = 4
            ps0 = [
                [
                    psum_pool.tile(
                        [P, N], mybir.dt.float32, name=f"ps{m}_{cc}", tag="ps"
                    )
                    for cc in range(2)
                ]
                for m in range(LEAD)
            ]
            for k in range(K_T):
                t = xwk[k]
                for m in range(LEAD):
                    mm(ps0[m][0], t, m, 0, k)
                    mm(ps0[m][1], t, m, 1, k)
                    if k == K_T - 1:
                        zh = z_pool.tile([P, C], out_mydt, name=f"z{m}", tag="z")
                        evict_pair(zh, ps0[m][0], ps0[m][1])
                        store_full(zh, m)

            # --- trail pass (rows m{LEAD}..m7): m-major, one full-row tile
            # retiring every ~3.5us.  The final tile is evicted and stored
            # as two half-row DMAs on parallel engines to shorten the tail.
            for m in range(LEAD, M_T):
                psa = psum_pool.tile([P, N], mybir.dt.float32, name=f"ps{m}_0", tag="ps")
                psb = psum_pool.tile([P, N], mybir.dt.float32, name=f"ps{m}_1", tag="ps")
                for k in range(K_T):
                    t = xwk[k]
                    mm(psa, t, m, 0, k)
                    mm(psb, t, m, 1, k)
                zh = z_pool.tile([P, C], out_mydt, name=f"z{m}", tag="z")
                evict_pair(zh, psa, psb)
                if m == M_T - 1:
                    nc.sync.dma_start(
                        out_rep[m * P : (m + 1) * P, :, :N], _rep3_src(zh[:, :N])
                    )
                    nc.scalar.dma_start(
                        out_rep[m * P : (m + 1) * P, :, N:], _rep3_src(zh[:, N:])
                    )
                else:
                    store_full(zh, m)

    _split_multiwaits(nc)
    return nc


_NC_CACHE: dict = {}


def _get_nc(compute_dt: str, out_dt: str) -> bass.Bass:
    key = (compute_dt, out_dt, N_WARM, M_LEAD)
    if key not in _NC_CACHE:
        _NC_CACHE[key] = _build(compute_dt, out_dt)
    return _NC_CACHE[key]


def kernel(x_q, x_kv, Wq, Wk, Wv, Wproj, _compute_dt=None, _out_dt=None):
    compute_dt = _compute_dt or COMPUTE_DT
    out_dt = _out_dt or OUT_DT
    B, Tkv, C_ = x_kv.shape
    assert (B, Tkv, C_) == (4, 2048, C)

    # Fold the two projections: z = x @ Wv.T @ Wproj.T = x @ WfT
    WfT = (Wv.astype(np.float64).T @ Wproj.astype(np.float64).T).astype(np.float32)

    x_flat = x_kv.reshape(B * Tkv, C)
    in_maps = []
    for c in range(N_CORES):
        shard = x_flat[c * ROWS_PER_CORE : (c + 1) * ROWS_PER_CORE]
        xw = np.concatenate([shard.T, WfT], axis=1)  # [C(k), 2048]
        if compute_dt == "bf16":
            import ml_dtypes

            xw = xw.astype(ml_dtypes.bfloat16)
        else:
            xw = np.ascontiguousarray(xw)
        in_maps.append({"xw": xw})

    nc = _get_nc(compute_dt, out_dt)
    res = run_bass_kernel_spmd(nc, in_maps, core_ids=list(range(N_CORES)))

    Tq = L * Tkv
    blocks = [res.results[c]["out"] for c in range(N_CORES)]
    out_flat = np.concatenate(blocks, axis=0)  # [B*Tq, C]
    return out_flat.reshape(B, Tq, C).astype(np.float32)
